# revision 1
# baseline (speedup 1.0000x reference)
"""LogGaborConv2d on 8 TRN2 NeuronCores.

Strategy: data-parallel over batch (8 images -> 8 cores). Per core:
- Gabor weights [O=128, I=64, 3, 3] computed on device from the params.
- 3x3 conv as 9 accumulating matmuls (K=64 input channels) over a
  column-padded flat image stream (width 258), windows of 512 pixels
  into PSUM banks.
- The 128 PE rows are split into two row-groups: partitions 0:64
  process the top half of the image, partitions 64:128 the bottom half,
  as concurrent K=64 matmuls (tile_position row groups), doubling PE
  throughput vs a single K=64 stream.
- fp32r matmul dtype: full-rate (1 cycle/row) with ~1e-4 relative error.

Host side only pads/shards inputs and de-pads/gathers outputs.
"""
import math

import numpy as np

import concourse.bacc as bacc
import concourse.bass as bass  # noqa: F401
import concourse.mybir as mybir
import concourse.tile as tile
from concourse.bass_utils import run_bass_kernel_spmd

F32 = mybir.dt.float32
F32R = mybir.dt.float32r
AF = mybir.ActivationFunctionType
OP = mybir.AluOpType

# problem constants
NB, C, H, W = 8, 64, 256, 256
O = 128
WP = W + 2            # padded row width
SL = (H + 2) * WP     # padded input stream length (incl. top/bottom pad rows)
OL = H * WP           # padded output stream length
NWIN = OL // 512      # 129 windows of 512
GUARD = 4             # leading guard zeros in the host-side stream
TLEN = 512 * 8 + 524  # input tile covers 8 windows + halo
TLEN_MINI = 512 + 524
XLEN = 512 * 128 + TLEN_MINI + GUARD  # 66572+4 -> round up
XLEN = (XLEN + 15) // 16 * 16
# grid values from reference: linspace(-1, 2, 3) both axes
_GRID = (-1.0, 0.5, 2.0)
DELTA = 0.001
NW_A = 64             # windows handled by partitions 0:64
# windows NW_A..128 handled by partitions 64:128


def _taps():
    """(tap_index, ky, kx, delta, r, exp_scale) for the 9 taps."""
    out = []
    for ky in range(3):
        for kx in range(3):
            t = 3 * ky + kx
            delta = ky * WP + (kx - 1)
            r2 = _GRID[kx] ** 2 + _GRID[ky] ** 2 + DELTA
            r = math.sqrt(r2)
            esc = -(math.log(r) ** 2) / 4.0
            out.append((t, ky, kx, delta, r, esc))
    return out


def build_kernel():
    nc = bacc.Bacc("TRN2", target_bir_lowering=False)
    x = nc.dram_tensor("x", [C, XLEN], F32R, kind="ExternalInput")
    params = nc.dram_tensor("params", [C, 512], F32, kind="ExternalInput")
    y = nc.dram_tensor("y", [O, OL], F32, kind="ExternalOutput")

    taps = _taps()

    with tile.TileContext(nc) as tc:
        with (
            tc.tile_pool(name="wg", bufs=1) as wg,
            tc.tile_pool(name="xin", bufs=2) as xin,
            tc.tile_pool(name="outp", bufs=3) as outp,
            tc.tile_pool(name="ps", bufs=2, space="PSUM") as ps,
        ):
            # ---------------- weight generation ----------------
            par = wg.tile([C, 512], F32)
            nc.sync.dma_start(par[:], params[:])
            th = par[:, 0:128]
            sg = par[:, 128:256]
            fr = par[:, 256:384]
            pss = par[:, 384:512]

            lnsg = wg.tile([C, 128], F32)
            nc.scalar.activation(lnsg[:], sg, AF.Ln)
            lsq = wg.tile([C, 128], F32)
            nc.vector.tensor_mul(lsq[:], lnsg[:], lnsg[:])
            il2 = wg.tile([C, 128], F32)
            nc.vector.reciprocal(il2[:], lsq[:])
            sg2 = wg.tile([C, 128], F32)
            nc.vector.tensor_mul(sg2[:], sg, sg)
            sinv = wg.tile([C, 128], F32)
            nc.vector.reciprocal(sinv[:], sg2[:])
            thm1 = wg.tile([C, 128], F32)
            nc.vector.tensor_scalar(thm1[:], th, 1.0, None, OP.subtract)
            a2 = wg.tile([C, 128], F32)
            nc.vector.tensor_mul(a2[:], thm1[:], thm1[:])
            asv = wg.tile([C, 128], F32)
            nc.vector.tensor_mul(asv[:], a2[:], sinv[:])
            e2 = wg.tile([C, 128], F32)
            nc.scalar.activation(e2[:], asv[:], AF.Exp, scale=-0.5)
            m1 = wg.tile([C, 128], F32)
            nc.vector.scalar_tensor_tensor(
                m1[:], e2[:], 1.0 / (2.0 * math.pi), sinv[:], OP.mult, OP.mult
            )

            argb = wg.tile([C, 1152], F32)
            eb = wg.tile([C, 1152], F32)
            for t, ky, kx, delta, r, esc in taps:
                nc.vector.scalar_tensor_tensor(
                    argb[:, 128 * t : 128 * t + 128], fr, float(r), pss,
                    OP.mult, OP.add,
                )
                nc.vector.tensor_scalar(
                    eb[:, 128 * t : 128 * t + 128], il2[:], float(esc), None,
                    OP.mult,
                )
            # cos(v) = sin(pi/2 - v), folded into [-pi, pi]
            wv = wg.tile([C, 1152], F32)
            nc.vector.tensor_scalar(
                wv[:], argb[:], -1.0, math.pi / 2.0, OP.mult, OP.add
            )
            msk = wg.tile([C, 1152], F32)
            nc.vector.tensor_single_scalar(msk[:], wv[:], -math.pi, OP.is_lt)
            wv2 = wg.tile([C, 1152], F32)
            nc.vector.scalar_tensor_tensor(
                wv2[:], msk[:], 2.0 * math.pi, wv[:], OP.mult, OP.add
            )
            cosb = wg.tile([C, 1152], F32)
            nc.scalar.activation(cosb[:], wv2[:], AF.Sin)
            e1b = wg.tile([C, 1152], F32)
            nc.scalar.activation(e1b[:], eb[:], AF.Exp)
            ecb = wg.tile([C, 1152], F32)
            nc.vector.tensor_mul(ecb[:], e1b[:], cosb[:])
            wt = wg.tile([O, 1152], F32R)
            for t, ky, kx, delta, r, esc in taps:
                nc.vector.tensor_mul(
                    wt[0:C, 128 * t : 128 * t + 128],
                    ecb[:, 128 * t : 128 * t + 128],
                    m1[:],
                )
            # duplicate weights into partitions 64:128 for the B row-group
            nc.sync.dma_start(wt[C : 2 * C, :], wt[0:C, :])

            # ---------------- convolution ----------------
            def emit_group(wa0, na, wb0, nb, xt, w0a, w0b):
                pa = [
                    ps.tile([O, 512], F32, tag=f"a{j}", name=f"pa{j}")
                    for j in range(na)
                ]
                pb = [
                    ps.tile([O, 512], F32, tag=f"b{j}", name=f"pb{j}")
                    for j in range(nb)
                ]
                ntap = len(taps)
                for t, ky, kx, delta, r, esc in taps:
                    lhs_a = wt[0:C, 128 * t : 128 * t + 128]
                    lhs_b = wt[C : 2 * C, 128 * t : 128 * t + 128]
                    first = t == 0
                    last = t == ntap - 1
                    for j in range(max(na, nb)):
                        if j < na:
                            o = 512 * (wa0 + j - w0a) + delta + GUARD
                            nc.tensor.matmul(
                                pa[j][:], lhs_a, xt[0:C, o : o + 512],
                                start=first, stop=last,
                            )
                        if j < nb:
                            o = 512 * (wb0 + j - w0b) + delta + GUARD
                            nc.tensor.matmul(
                                pb[j][:], lhs_b, xt[C : 2 * C, o : o + 512],
                                start=first, stop=last,
                            )
                ot = outp.tile([O, 512 * (na + nb)], F32, tag="ot", name="ot")
                for j in range(na):
                    eng = nc.scalar if j % 2 == 0 else nc.vector
                    if eng is nc.scalar:
                        nc.scalar.copy(ot[:, 512 * j : 512 * j + 512], pa[j][:])
                    else:
                        nc.vector.tensor_copy(
                            ot[:, 512 * j : 512 * j + 512], pa[j][:]
                        )
                for j in range(nb):
                    c0 = 512 * (na + j)
                    if j % 2 == 1:
                        nc.scalar.copy(ot[:, c0 : c0 + 512], pb[j][:])
                    else:
                        nc.vector.tensor_copy(ot[:, c0 : c0 + 512], pb[j][:])
                if na:
                    nc.sync.dma_start(
                        y[:, 512 * wa0 : 512 * (wa0 + na)], ot[:, 0 : 512 * na]
                    )
                if nb:
                    nc.sync.dma_start(
                        y[:, 512 * wb0 : 512 * (wb0 + nb)],
                        ot[:, 512 * na : 512 * (na + nb)],
                    )

            for tblk in range(8):
                w0a = 8 * tblk
                w0b = NW_A + 8 * tblk
                xt = xin.tile([2 * C, TLEN], F32R, tag="xt", name="xt")
                nc.sync.dma_start(
                    xt[0:C, :], x[:, 512 * w0a : 512 * w0a + TLEN]
                )
                nc.sync.dma_start(
                    xt[C : 2 * C, :], x[:, 512 * w0b : 512 * w0b + TLEN]
                )
                for sub in range(4):
                    emit_group(
                        w0a + 2 * sub, 2, w0b + 2 * sub, 2, xt, w0a, w0b
                    )
            # final window 128 on the B row-group
            xtm = xin.tile([2 * C, TLEN], F32R, tag="xt", name="xtm")
            nc.sync.dma_start(
                xtm[C : 2 * C, 0:TLEN_MINI],
                x[:, 512 * 128 : 512 * 128 + TLEN_MINI],
            )
            emit_group(0, 0, 128, 1, xtm, 0, 128)

    nc.compile()
    return nc


_NC_CACHE = None


def _get_nc():
    global _NC_CACHE
    if _NC_CACHE is None:
        _NC_CACHE = build_kernel()
    return _NC_CACHE


def kernel(input_tensor, freq, theta, sigma, psi, f0, theta0, xg, yg):
    x = np.ascontiguousarray(np.asarray(input_tensor, dtype=np.float32))
    params = np.ascontiguousarray(
        np.concatenate(
            [
                np.asarray(theta, np.float32).T,
                np.asarray(sigma, np.float32).T,
                np.asarray(freq, np.float32).T,
                np.asarray(psi, np.float32).T,
            ],
            axis=1,
        )
    )
    nc = _get_nc()
    in_maps = []
    for c in range(NB):
        xp = np.zeros((C, XLEN), np.float32)
        view = xp[:, GUARD : GUARD + SL].reshape(C, H + 2, WP)
        view[:, 1 : H + 1, 1 : W + 1] = x[c]
        in_maps.append({"x": xp, "params": params})
    res = run_bass_kernel_spmd(nc, in_maps, core_ids=list(range(NB)))
    out = np.empty((NB, O, H, W), np.float32)
    for c in range(NB):
        out[c] = res.results[c]["y"].reshape(O, H, WP)[:, :, 1 : W + 1]
    return out



# revision 5
# speedup vs baseline: 1.3481x; 1.3481x over previous
"""LogGaborConv2d on 8 TRN2 NeuronCores.

Strategy: data-parallel over batch (8 images -> 8 cores). Per core:
- Gabor weights [O=128, I=64, 3, 3] computed on the host (tiny) and fed
  to the device as an fp16 [128, 768] matrix.
- Input is staged as an fp16 column-padded flat stream (width 258) in
  two partition halves: partitions 0:64 hold the stream, partitions
  64:128 hold the same stream shifted by one image row (+258). This
  lets taps (ky=0,kx) and (ky=1,kx) fuse into a single K=128 matmul
  using the full 128x128 PE array. The leftover ky=2 taps run as K=64
  matmuls co-executed pairwise across adjacent windows via
  tile_position row groups.
- Tap-outer ordering over 4-window blocks (4 PSUM banks, double
  buffered) keeps weight loads coherent and the PE queue short.
- fp16 streams everywhere; PSUM accumulates in fp32; output written
  back as fp16 and upconverted on the host.
"""
import math

import numpy as np

import concourse.bacc as bacc
import concourse.bass as bass  # noqa: F401
import concourse.mybir as mybir
import concourse.tile as tile
from concourse.bass_utils import run_bass_kernel_spmd

F32 = mybir.dt.float32
F16 = mybir.dt.float16

# problem constants
NB, C, H, W = 8, 64, 256, 256
O = 128
WP = W + 2            # padded row width
SL = (H + 2) * WP     # padded input stream length (incl. top/bottom pad rows)
OL = H * WP           # padded output stream length: 129 windows of 512
GUARD = 4             # leading guard zeros in the host-side stream
XLEN = 66592          # >= GUARD + SL + slack, mult of 16
TLEN = 512 * 8 + 528  # input tile: 8 windows + halo (517+511 -> 1028 cols max)
TLEN_MINI = 1040      # final window tile
DELTA = 0.001


def build_kernel():
    nc = bacc.Bacc("TRN2", target_bir_lowering=False)
    x = nc.dram_tensor("x", [2 * C, XLEN], F16, kind="ExternalInput")
    w = nc.dram_tensor("w", [2 * C, 768], F16, kind="ExternalInput")
    y = nc.dram_tensor("y", [O, OL], F16, kind="ExternalOutput")

    with tile.TileContext(nc) as tc:
        with (
            tc.tile_pool(name="wg", bufs=1) as wg,
            tc.tile_pool(name="xin", bufs=2) as xin,
            tc.tile_pool(name="outp", bufs=3) as outp,
            tc.tile_pool(name="ps", bufs=2, space="PSUM") as ps,
        ):
            wt = wg.tile([2 * C, 768], F16)
            nc.sync.dma_start(wt[:], w[:])

            def copy_engine(i):
                return (nc.scalar.copy, nc.vector.tensor_copy)[i % 2]

            def emit_block(pt, xt, q0):
                """4 windows starting at in-tile col q0 (window stride 512).

                pt: list of 4 PSUM tiles. Tap-outer: 3 fused K=128 configs,
                then 3 solo K=64 configs co-executed A/B.
                """
                for kx in range(3):
                    lhs = wt[0 : 2 * C, 128 * kx : 128 * kx + 128]
                    for j in range(4):
                        o = q0 + 512 * j + kx - 1
                        nc.tensor.matmul(
                            pt[j][:], lhs, xt[0 : 2 * C, o : o + 512],
                            start=(kx == 0), stop=False,
                            tile_position=(0, 0),
                        )
                for kx in range(3):
                    lhs_a = wt[0:C, 384 + 128 * kx : 384 + 128 * kx + 128]
                    lhs_b = wt[C : 2 * C, 384 + 128 * kx : 384 + 128 * kx + 128]
                    last = kx == 2
                    for j in range(4):
                        o = q0 + 512 * j + 258 + kx - 1
                        if j % 2 == 0:
                            # A row group reads copy1 at +516 total
                            nc.tensor.matmul(
                                pt[j][:], lhs_a,
                                xt[0:C, o + 258 : o + 258 + 512],
                                start=False, stop=last,
                                tile_position=(0, 0),
                            )
                        else:
                            # B row group reads copy2 (=stream+258) at +258
                            nc.tensor.matmul(
                                pt[j][:], lhs_b,
                                xt[C : 2 * C, o : o + 512],
                                start=False, stop=last,
                                tile_position=(64, 0),
                            )

            for tblk in range(16):
                w0 = 8 * tblk
                xt = xin.tile([2 * C, TLEN], F16, tag="xt", name="xt")
                nc.sync.dma_start(xt[:], x[:, 512 * w0 : 512 * w0 + TLEN])
                ot = outp.tile([O, 4096], F16, tag="ot", name="ot")
                for blk in range(2):
                    pt = [
                        ps.tile([O, 512], F32, tag=f"p{j}", name=f"p{j}")
                        for j in range(4)
                    ]
                    emit_block(pt, xt, GUARD + 2048 * blk)
                    for j in range(4):
                        c0 = 2048 * blk + 512 * j
                        copy_engine(4 * blk + j)(
                            ot[:, c0 : c0 + 512], pt[j][:]
                        )
                nc.sync.dma_start(y[:, 512 * w0 : 512 * w0 + 4096], ot[:])

            # final window 128 (unpaired): 3 fused + 3 solo on row group A
            xtm = xin.tile([2 * C, TLEN_MINI], F16, tag="xtm", name="xtm")
            nc.sync.dma_start(
                xtm[:], x[:, 512 * 128 : 512 * 128 + TLEN_MINI]
            )
            pm = ps.tile([O, 512], F32, tag="p0", name="pm")
            for kx in range(3):
                nc.tensor.matmul(
                    pm[:], wt[0 : 2 * C, 128 * kx : 128 * kx + 128],
                    xtm[0 : 2 * C, GUARD + kx - 1 : GUARD + kx - 1 + 512],
                    start=(kx == 0), stop=False, tile_position=(0, 0),
                )
            for kx in range(3):
                o = GUARD + 516 + kx - 1
                nc.tensor.matmul(
                    pm[:], wt[0:C, 384 + 128 * kx : 384 + 128 * kx + 128],
                    xtm[0:C, o : o + 512],
                    start=False, stop=(kx == 2), tile_position=(0, 0),
                )
            om = outp.tile([O, 512], F16, tag="om", name="om")
            nc.scalar.copy(om[:], pm[:])
            nc.sync.dma_start(y[:, 512 * 128 : 512 * 129], om[:])

    nc.compile()
    return nc


_NC_CACHE = None


def _get_nc():
    global _NC_CACHE
    if _NC_CACHE is None:
        _NC_CACHE = build_kernel()
    return _NC_CACHE


def _gabor_weights(freq, theta, sigma, psi, f0, theta0, xg, yg):
    """[O, I, 3, 3] float32, matching the reference math."""
    th = theta[:, :, None, None].astype(np.float64)
    sg = sigma[:, :, None, None].astype(np.float64)
    fr = freq[:, :, None, None].astype(np.float64)
    ps = psi[:, :, None, None].astype(np.float64)
    xgd = xg.astype(np.float64)
    ygd = yg.astype(np.float64)
    lf0 = math.log(float(f0[0]))
    th0 = float(theta0[0])
    # rotation preserves radius
    r = np.sqrt(xgd**2 + ygd**2 + DELTA)[None, None]
    g_radial = np.exp(-((np.log(r) - lf0) / (2.0 * (np.log(sg) - lf0))) ** 2)
    g_angular = np.exp(-((th - th0) ** 2) / (2.0 * sg**2))
    g = g_radial * g_angular * np.cos(fr * r + ps) / (2.0 * math.pi * sg**2)
    return g.astype(np.float32)


def kernel(input_tensor, freq, theta, sigma, psi, f0, theta0, xg, yg):
    wfull = _gabor_weights(freq, theta, sigma, psi, f0, theta0, xg, yg)
    wmat = np.zeros((2 * C, 768), np.float16)
    for kx in range(3):
        wmat[0:C, 128 * kx : 128 * kx + 128] = wfull[:, :, 0, kx].T
        wmat[C : 2 * C, 128 * kx : 128 * kx + 128] = wfull[:, :, 1, kx].T
        wmat[0:C, 384 + 128 * kx : 384 + 128 * kx + 128] = wfull[:, :, 2, kx].T
        wmat[C : 2 * C, 384 + 128 * kx : 384 + 128 * kx + 128] = (
            wfull[:, :, 2, kx].T
        )

    x16 = np.asarray(input_tensor, dtype=np.float16)
    nc = _get_nc()
    in_maps = []
    for c in range(NB):
        xp = np.zeros((2 * C, XLEN), np.float16)
        view = xp[0:C, GUARD : GUARD + SL].reshape(C, H + 2, WP)
        view[:, 1 : H + 1, 1 : W + 1] = x16[c]
        xp[C : 2 * C, 0 : XLEN - WP] = xp[0:C, WP:XLEN]
        in_maps.append({"x": xp, "w": wmat})
    res = run_bass_kernel_spmd(nc, in_maps, core_ids=list(range(NB)))
    out = np.empty((NB, O, H, W), np.float32)
    for c in range(NB):
        out[c] = (
            res.results[c]["y"]
            .reshape(O, H, WP)[:, :, 1 : W + 1]
            .astype(np.float32)
        )
    return out


# revision 8
# speedup vs baseline: 1.4039x; 1.0414x over previous
"""LogGaborConv2d on 8 TRN2 NeuronCores.

Strategy: data-parallel over batch (8 images -> 8 cores). Per core:
- Gabor weights [O=128, I=64, 3, 3] computed on the host (tiny) and fed
  to the device as an fp16 [128, 768] matrix.
- Input is staged as an fp16 column-padded flat stream (width 258) in
  two partition halves: partitions 0:64 hold the stream, partitions
  64:128 hold the same stream shifted by one image row (+258). This
  lets taps (ky=0,kx) and (ky=1,kx) fuse into a single K=128 matmul
  using the full 128x128 PE array. The leftover ky=2 taps run as K=64
  matmuls co-executed pairwise across adjacent windows via
  tile_position row groups.
- Tap-outer ordering over 4-window blocks (4 PSUM banks, double
  buffered) keeps weight loads coherent and the PE queue short.
- fp16 streams everywhere; PSUM accumulates in fp32; output written
  back as fp16 and upconverted on the host.
"""
import math

import numpy as np

import concourse.bacc as bacc
import concourse.bass as bass  # noqa: F401
import concourse.mybir as mybir
import concourse.tile as tile
from concourse.bass_utils import run_bass_kernel_spmd

F32 = mybir.dt.float32
F16 = mybir.dt.float16

# problem constants
NB, C, H, W = 8, 64, 256, 256
O = 128
WP = W + 2            # padded row width
SL = (H + 2) * WP     # padded input stream length (incl. top/bottom pad rows)
OL = H * WP           # padded output stream length: 129 windows of 512
GUARD = 4             # leading guard zeros in the host-side stream
XLEN = 66592          # >= GUARD + SL + slack, mult of 16
TLEN = 512 * 8 + 528  # input tile: 8 windows + halo (517+511 -> 1028 cols max)
TLEN_MINI = 1040      # final window tile
DELTA = 0.001


def build_kernel():
    nc = bacc.Bacc("TRN2", target_bir_lowering=False)
    x = nc.dram_tensor("x", [2 * C, XLEN], F16, kind="ExternalInput")
    w = nc.dram_tensor("w", [2 * C, 768], F16, kind="ExternalInput")
    y = nc.dram_tensor("y", [O, OL], F16, kind="ExternalOutput")

    with tile.TileContext(nc) as tc:
        with (
            tc.tile_pool(name="wg", bufs=1) as wg,
            tc.tile_pool(name="xin", bufs=3) as xin,
            tc.tile_pool(name="outp", bufs=3) as outp,
            tc.tile_pool(name="ps", bufs=2, space="PSUM") as ps,
        ):
            wt = wg.tile([2 * C, 768], F16)
            nc.sync.dma_start(wt[:], w[:])

            def copy_engine(i):
                return (nc.scalar.copy, nc.vector.tensor_copy)[i % 2]

            def emit_block(pt, xt, q0):
                """4 windows starting at in-tile col q0 (window stride 512).

                pt: list of 4 PSUM tiles. Tap-outer: 3 fused K=128 configs,
                then 3 solo K=64 configs co-executed A/B.
                """
                for kx in range(3):
                    lhs = wt[0 : 2 * C, 128 * kx : 128 * kx + 128]
                    for j in range(4):
                        o = q0 + 512 * j + kx - 1
                        nc.tensor.matmul(
                            pt[j][:], lhs, xt[0 : 2 * C, o : o + 512],
                            start=(kx == 0), stop=False,
                            tile_position=(0, 0),
                        )
                for kx in range(3):
                    lhs_a = wt[0:C, 384 + 128 * kx : 384 + 128 * kx + 128]
                    lhs_b = wt[C : 2 * C, 384 + 128 * kx : 384 + 128 * kx + 128]
                    last = kx == 2
                    for j in range(4):
                        o = q0 + 512 * j + 258 + kx - 1
                        if j % 2 == 0:
                            # A row group reads copy1 at +516 total
                            nc.tensor.matmul(
                                pt[j][:], lhs_a,
                                xt[0:C, o + 258 : o + 258 + 512],
                                start=False, stop=last,
                                tile_position=(0, 0),
                            )
                        else:
                            # B row group reads copy2 (=stream+258) at +258
                            nc.tensor.matmul(
                                pt[j][:], lhs_b,
                                xt[C : 2 * C, o : o + 512],
                                start=False, stop=last,
                                tile_position=(64, 0),
                            )

            CHUNK = 2048 + 592  # covers block 0's span incl. halo
            for tblk in range(16):
                w0 = 8 * tblk
                xt = xin.tile([2 * C, TLEN], F16, tag="xt", name="xt")
                if tblk == 0:
                    # split the first load so block 0 can start early
                    nc.sync.dma_start(
                        xt[:, 0:CHUNK], x[:, 512 * w0 : 512 * w0 + CHUNK]
                    )
                    nc.sync.dma_start(
                        xt[:, CHUNK:TLEN],
                        x[:, 512 * w0 + CHUNK : 512 * w0 + TLEN],
                    )
                else:
                    nc.sync.dma_start(xt[:], x[:, 512 * w0 : 512 * w0 + TLEN])
                ot = outp.tile([O, 4096], F16, tag="ot", name="ot")
                for blk in range(2):
                    pt = [
                        ps.tile([O, 512], F32, tag=f"p{j}", name=f"p{j}")
                        for j in range(4)
                    ]
                    emit_block(pt, xt, GUARD + 2048 * blk)
                    for j in range(4):
                        c0 = 2048 * blk + 512 * j
                        copy_engine(4 * blk + j)(
                            ot[:, c0 : c0 + 512], pt[j][:]
                        )
                    nc.gpsimd.dma_start(
                        y[:, 512 * w0 + 2048 * blk : 512 * w0 + 2048 * blk + 2048],
                        ot[:, 2048 * blk : 2048 * blk + 2048],
                    )

            # final window 128 (unpaired): 3 fused + 3 solo on row group A
            xtm = xin.tile([2 * C, TLEN_MINI], F16, tag="xtm", name="xtm")
            nc.sync.dma_start(
                xtm[:], x[:, 512 * 128 : 512 * 128 + TLEN_MINI]
            )
            pm = ps.tile([O, 512], F32, tag="p0", name="pm")
            for kx in range(3):
                nc.tensor.matmul(
                    pm[:], wt[0 : 2 * C, 128 * kx : 128 * kx + 128],
                    xtm[0 : 2 * C, GUARD + kx - 1 : GUARD + kx - 1 + 512],
                    start=(kx == 0), stop=False, tile_position=(0, 0),
                )
            for kx in range(3):
                o = GUARD + 516 + kx - 1
                nc.tensor.matmul(
                    pm[:], wt[0:C, 384 + 128 * kx : 384 + 128 * kx + 128],
                    xtm[0:C, o : o + 512],
                    start=False, stop=(kx == 2), tile_position=(0, 0),
                )
            om = outp.tile([O, 512], F16, tag="om", name="om")
            nc.scalar.copy(om[:], pm[:])
            nc.gpsimd.dma_start(y[:, 512 * 128 : 512 * 129], om[:])

    nc.compile()
    return nc


_NC_CACHE = None


def _get_nc():
    global _NC_CACHE
    if _NC_CACHE is None:
        _NC_CACHE = build_kernel()
    return _NC_CACHE


def _gabor_weights(freq, theta, sigma, psi, f0, theta0, xg, yg):
    """[O, I, 3, 3] float32, matching the reference math."""
    th = theta[:, :, None, None].astype(np.float64)
    sg = sigma[:, :, None, None].astype(np.float64)
    fr = freq[:, :, None, None].astype(np.float64)
    ps = psi[:, :, None, None].astype(np.float64)
    xgd = xg.astype(np.float64)
    ygd = yg.astype(np.float64)
    lf0 = math.log(float(f0[0]))
    th0 = float(theta0[0])
    # rotation preserves radius
    r = np.sqrt(xgd**2 + ygd**2 + DELTA)[None, None]
    g_radial = np.exp(-((np.log(r) - lf0) / (2.0 * (np.log(sg) - lf0))) ** 2)
    g_angular = np.exp(-((th - th0) ** 2) / (2.0 * sg**2))
    g = g_radial * g_angular * np.cos(fr * r + ps) / (2.0 * math.pi * sg**2)
    return g.astype(np.float32)


def kernel(input_tensor, freq, theta, sigma, psi, f0, theta0, xg, yg):
    wfull = _gabor_weights(freq, theta, sigma, psi, f0, theta0, xg, yg)
    wmat = np.zeros((2 * C, 768), np.float16)
    for kx in range(3):
        wmat[0:C, 128 * kx : 128 * kx + 128] = wfull[:, :, 0, kx].T
        wmat[C : 2 * C, 128 * kx : 128 * kx + 128] = wfull[:, :, 1, kx].T
        wmat[0:C, 384 + 128 * kx : 384 + 128 * kx + 128] = wfull[:, :, 2, kx].T
        wmat[C : 2 * C, 384 + 128 * kx : 384 + 128 * kx + 128] = (
            wfull[:, :, 2, kx].T
        )

    x16 = np.asarray(input_tensor, dtype=np.float16)
    nc = _get_nc()
    in_maps = []
    for c in range(NB):
        xp = np.zeros((2 * C, XLEN), np.float16)
        view = xp[0:C, GUARD : GUARD + SL].reshape(C, H + 2, WP)
        view[:, 1 : H + 1, 1 : W + 1] = x16[c]
        xp[C : 2 * C, 0 : XLEN - WP] = xp[0:C, WP:XLEN]
        in_maps.append({"x": xp, "w": wmat})
    res = run_bass_kernel_spmd(nc, in_maps, core_ids=list(range(NB)))
    out = np.empty((NB, O, H, W), np.float32)
    for c in range(NB):
        out[c] = (
            res.results[c]["y"]
            .reshape(O, H, WP)[:, :, 1 : W + 1]
            .astype(np.float32)
        )
    return out


# revision 9
# speedup vs baseline: 1.4063x; 1.0017x over previous
"""LogGaborConv2d on 8 TRN2 NeuronCores.

Strategy: data-parallel over batch (8 images -> 8 cores). Per core:
- Gabor weights [O=128, I=64, 3, 3] computed on the host (tiny) and fed
  to the device as an fp16 [128, 768] matrix.
- Input is staged as an fp16 column-padded flat stream (width 258) in
  two partition halves: partitions 0:64 hold the stream, partitions
  64:128 hold the same stream shifted by one image row (+258). This
  lets taps (ky=0,kx) and (ky=1,kx) fuse into a single K=128 matmul
  using the full 128x128 PE array. The leftover ky=2 taps run as K=64
  matmuls co-executed pairwise across adjacent windows via
  tile_position row groups.
- Tap-outer ordering over 4-window blocks (4 PSUM banks, double
  buffered) keeps weight loads coherent and the PE queue short.
- fp16 streams everywhere; PSUM accumulates in fp32; output written
  back as fp16 and upconverted on the host.
"""
import math

import numpy as np

import concourse.bacc as bacc
import concourse.bass as bass  # noqa: F401
import concourse.mybir as mybir
import concourse.tile as tile
from concourse.bass_utils import run_bass_kernel_spmd

F32 = mybir.dt.float32
F16 = mybir.dt.float16

# problem constants
NB, C, H, W = 8, 64, 256, 256
O = 128
WP = W + 2            # padded row width
SL = (H + 2) * WP     # padded input stream length (incl. top/bottom pad rows)
OL = H * WP           # padded output stream length: 129 windows of 512
GUARD = 4             # leading guard zeros in the host-side stream
XLEN = 66592          # >= GUARD + SL + slack, mult of 16
TLEN = 512 * 8 + 528  # input tile: 8 windows + halo (517+511 -> 1028 cols max)
TLEN_MINI = 1040      # final window tile
DELTA = 0.001


def build_kernel():
    nc = bacc.Bacc("TRN2", target_bir_lowering=False)
    x = nc.dram_tensor("x", [2 * C, XLEN], F16, kind="ExternalInput")
    w = nc.dram_tensor("w", [2 * C, 768], F16, kind="ExternalInput")
    y = nc.dram_tensor("y", [O, OL], F16, kind="ExternalOutput")

    with tile.TileContext(nc) as tc:
        with (
            tc.tile_pool(name="wg", bufs=1) as wg,
            tc.tile_pool(name="xin", bufs=3) as xin,
            tc.tile_pool(name="outp", bufs=3) as outp,
            tc.tile_pool(name="ps", bufs=2, space="PSUM") as ps,
        ):
            wt = wg.tile([2 * C, 768], F16)
            nc.sync.dma_start(wt[:], w[:])

            def copy_engine(i):
                return (nc.scalar.copy, nc.vector.tensor_copy)[i % 2]

            def emit_block(pt, xt, q0):
                """4 windows starting at in-tile col q0 (window stride 512).

                pt: list of 4 PSUM tiles. Tap-outer: 3 fused K=128 configs,
                then 3 solo K=64 configs co-executed A/B.
                """
                for kx in range(3):
                    lhs = wt[0 : 2 * C, 128 * kx : 128 * kx + 128]
                    for j in range(4):
                        o = q0 + 512 * j + kx - 1
                        nc.tensor.matmul(
                            pt[j][:], lhs, xt[0 : 2 * C, o : o + 512],
                            start=(kx == 0), stop=False,
                            tile_position=(0, 0),
                        )
                for kx in range(3):
                    lhs_a = wt[0:C, 384 + 128 * kx : 384 + 128 * kx + 128]
                    lhs_b = wt[C : 2 * C, 384 + 128 * kx : 384 + 128 * kx + 128]
                    last = kx == 2
                    for j in range(4):
                        o = q0 + 512 * j + 258 + kx - 1
                        if j % 2 == 0:
                            # A row group reads copy1 at +516 total
                            nc.tensor.matmul(
                                pt[j][:], lhs_a,
                                xt[0:C, o + 258 : o + 258 + 512],
                                start=False, stop=last,
                                tile_position=(0, 0),
                            )
                        else:
                            # B row group reads copy2 (=stream+258) at +258
                            nc.tensor.matmul(
                                pt[j][:], lhs_b,
                                xt[C : 2 * C, o : o + 512],
                                start=False, stop=last,
                                tile_position=(64, 0),
                            )

            def emit_mini():
                """final window 128 (unpaired): 3 fused + 3 solo on group A"""
                xtm = xin.tile([2 * C, TLEN], F16, tag="xt", name="xtm")
                nc.sync.dma_start(
                    xtm[:, 0:TLEN_MINI],
                    x[:, 512 * 128 : 512 * 128 + TLEN_MINI],
                )
                pm = ps.tile([O, 512], F32, tag="p0", name="pm")
                for kx in range(3):
                    nc.tensor.matmul(
                        pm[:], wt[0 : 2 * C, 128 * kx : 128 * kx + 128],
                        xtm[0 : 2 * C, GUARD + kx - 1 : GUARD + kx - 1 + 512],
                        start=(kx == 0), stop=False, tile_position=(0, 0),
                    )
                for kx in range(3):
                    o = GUARD + 516 + kx - 1
                    nc.tensor.matmul(
                        pm[:], wt[0:C, 384 + 128 * kx : 384 + 128 * kx + 128],
                        xtm[0:C, o : o + 512],
                        start=False, stop=(kx == 2), tile_position=(0, 0),
                    )
                om = outp.tile([O, 4096], F16, tag="ot", name="om")
                nc.scalar.copy(om[:, 0:512], pm[:])
                nc.gpsimd.dma_start(y[:, 512 * 128 : 512 * 129], om[:, 0:512])

            CH1, CH2 = 1104, 2640  # first-tile chunk boundaries
            for tblk in range(16):
                w0 = 8 * tblk
                xt = xin.tile([2 * C, TLEN], F16, tag="xt", name="xt")
                if tblk == 0:
                    # split the first load so block 0 can start early
                    nc.sync.dma_start(xt[:, 0:CH1], x[:, 0:CH1])
                    nc.sync.dma_start(xt[:, CH1:CH2], x[:, CH1:CH2])
                    nc.sync.dma_start(xt[:, CH2:TLEN], x[:, CH2:TLEN])
                else:
                    nc.sync.dma_start(xt[:], x[:, 512 * w0 : 512 * w0 + TLEN])
                ot = outp.tile([O, 4096], F16, tag="ot", name="ot")
                for blk in range(2):
                    pt = [
                        ps.tile([O, 512], F32, tag=f"p{j}", name=f"p{j}")
                        for j in range(4)
                    ]
                    emit_block(pt, xt, GUARD + 2048 * blk)
                    for j in range(4):
                        c0 = 2048 * blk + 512 * j
                        copy_engine(4 * blk + j)(
                            ot[:, c0 : c0 + 512], pt[j][:]
                        )
                nc.gpsimd.dma_start(y[:, 512 * w0 : 512 * w0 + 4096], ot[:])
                if tblk == 0:
                    emit_mini()

    nc.compile()
    return nc


_NC_CACHE = None


def _get_nc():
    global _NC_CACHE
    if _NC_CACHE is None:
        _NC_CACHE = build_kernel()
    return _NC_CACHE


def _gabor_weights(freq, theta, sigma, psi, f0, theta0, xg, yg):
    """[O, I, 3, 3] float32, matching the reference math."""
    th = theta[:, :, None, None].astype(np.float64)
    sg = sigma[:, :, None, None].astype(np.float64)
    fr = freq[:, :, None, None].astype(np.float64)
    ps = psi[:, :, None, None].astype(np.float64)
    xgd = xg.astype(np.float64)
    ygd = yg.astype(np.float64)
    lf0 = math.log(float(f0[0]))
    th0 = float(theta0[0])
    # rotation preserves radius
    r = np.sqrt(xgd**2 + ygd**2 + DELTA)[None, None]
    g_radial = np.exp(-((np.log(r) - lf0) / (2.0 * (np.log(sg) - lf0))) ** 2)
    g_angular = np.exp(-((th - th0) ** 2) / (2.0 * sg**2))
    g = g_radial * g_angular * np.cos(fr * r + ps) / (2.0 * math.pi * sg**2)
    return g.astype(np.float32)


def kernel(input_tensor, freq, theta, sigma, psi, f0, theta0, xg, yg):
    wfull = _gabor_weights(freq, theta, sigma, psi, f0, theta0, xg, yg)
    wmat = np.zeros((2 * C, 768), np.float16)
    for kx in range(3):
        wmat[0:C, 128 * kx : 128 * kx + 128] = wfull[:, :, 0, kx].T
        wmat[C : 2 * C, 128 * kx : 128 * kx + 128] = wfull[:, :, 1, kx].T
        wmat[0:C, 384 + 128 * kx : 384 + 128 * kx + 128] = wfull[:, :, 2, kx].T
        wmat[C : 2 * C, 384 + 128 * kx : 384 + 128 * kx + 128] = (
            wfull[:, :, 2, kx].T
        )

    x16 = np.asarray(input_tensor, dtype=np.float16)
    nc = _get_nc()
    in_maps = []
    for c in range(NB):
        xp = np.zeros((2 * C, XLEN), np.float16)
        view = xp[0:C, GUARD : GUARD + SL].reshape(C, H + 2, WP)
        view[:, 1 : H + 1, 1 : W + 1] = x16[c]
        xp[C : 2 * C, 0 : XLEN - WP] = xp[0:C, WP:XLEN]
        in_maps.append({"x": xp, "w": wmat})
    res = run_bass_kernel_spmd(nc, in_maps, core_ids=list(range(NB)))
    out = np.empty((NB, O, H, W), np.float32)
    for c in range(NB):
        out[c] = (
            res.results[c]["y"]
            .reshape(O, H, WP)[:, :, 1 : W + 1]
            .astype(np.float32)
        )
    return out
